# revision 1
# baseline (speedup 1.0000x reference)
"""GIN GNN kernel builder for 8 TRN2 NeuronCores (SPMD via run_bass_kernel_spmd).

v2: bf16 compute on all PE operands (single-pass matmuls + fast weight load),
batched per-tile S-matrix builds, degree-balanced node packing.

Sharding: graphs partitioned contiguously into 8 equal groups; each core owns
the contiguous node range of its graphs, padded to PAD_N rows. Edges routed to
the dst-owning core. Aggregation gathers from a replicated y tensor
(y = h_prev @ W1, exploiting GIN linearity), refreshed via AllGather between
layers. Aggregation itself: per 128-dst-node tile, chunks of 128 edges are
gathered (indirect DMA) and accumulated into PSUM via matmul with a one-hot
selection matrix S[p,j] = (dstslot[p]==j). BN+biases fold into per-partition
ACT scale/bias. The last matmul of each tile uses an lhsT-swap to produce
node-major output directly (no transposes anywhere). Pooling accumulates
pooledT[f,g] per 128-graph block in PSUM; the FC head runs per block.
"""

import sys

sys.path.insert(0, "/opt/trn_rl_repo")

import numpy as np
import concourse.bass as bass
import concourse.bacc as bacc
import concourse.mybir as mybir
import concourse.tile as tile
from concourse import bass_utils
from concourse.masks import make_identity

P = 128
BN_EPS = 1e-5
BF16 = np.float16


class Cfg:
    def __init__(self, n_nodes, n_edges, n_graphs, f_node, h, ncores, exch="bf16"):
        self.N, self.E, self.G, self.F, self.H = n_nodes, n_edges, n_graphs, f_node, h
        self.NCORES = ncores
        self.G_PER_CORE = n_graphs // ncores
        self.exch = exch


def _balance_nodes(deg, batch_local, gpc):
    """Per-128-graph-block LPT: equalize per-tile in-degree sums, moving nodes
    only within their block's position range. perm[i] = old index of the node
    placed at new position i."""
    n = len(deg)
    perm = np.arange(n)
    JB = int(np.ceil(gpc / P))
    bounds = np.searchsorted(batch_local, np.arange(JB + 1) * P, side="left")
    for J in range(JB):
        lo, hi = int(bounds[J]), int(min(bounds[J + 1], n))
        if hi - lo < 2:
            continue
        pos = np.arange(lo, hi)
        utiles = np.unique(pos // P)
        d = deg[lo:hi]
        order = np.argsort(-d, kind="stable")
        load = {int(t): 0.0 for t in utiles}
        slots = {int(t): [] for t in utiles}
        for p_ in pos:
            slots[int(p_ // P)].append(int(p_))
        ut = [int(t) for t in utiles]
        for i in order:
            t = min((tt for tt in ut if slots[tt]), key=lambda tt: load[tt])
            load[t] += d[i]
            perm[slots[t].pop()] = lo + i
    return perm


def preprocess_graph(cfg, edge_index, batch, balance=True):
    nco, gpc = cfg.NCORES, cfg.G_PER_CORE
    batch = np.asarray(batch)
    ei = np.asarray(edge_index)
    src, dst = ei[0].astype(np.int64), ei[1].astype(np.int64)

    node_start = np.searchsorted(batch, np.arange(nco + 1) * gpc, side="left")
    counts = np.diff(node_start)
    PAD_N = int(np.ceil(counts.max() / P) * P)
    NT = PAD_N // P

    deg = np.zeros(cfg.N, np.int64)
    np.add.at(deg, dst, 1)

    local = np.empty(cfg.N, np.int64)
    invperm_by_core = []
    for k in range(nco):
        s, e = node_start[k], node_start[k + 1]
        bl = batch[s:e] - k * gpc
        if balance:
            perm = _balance_nodes(deg[s:e], bl, gpc)
        else:
            perm = np.arange(e - s)
        inv = np.empty_like(perm)
        inv[perm] = np.arange(len(perm))  # old_idx -> new_pos
        local[s:e] = inv
        invperm_by_core.append(perm)

    core_of = np.searchsorted(node_start, np.arange(cfg.N), side="right") - 1
    gpad = core_of * PAD_N + local

    dcore = core_of[dst]
    dloc = local[dst]
    dtile = dloc // P
    dslot = dloc % P

    MW = 4  # tiles per macro window (512 PSUM slots)
    NM = (NT + MW - 1) // MW
    dmacro = dloc // (MW * P)
    mslot = dloc % (MW * P)
    cntm = np.zeros((nco, NM), np.int64)
    np.add.at(cntm, (dcore, dmacro), 1)
    c_per = np.ceil(cntm / P).astype(np.int64)
    C = np.maximum(c_per.max(axis=0), 1)
    off = np.concatenate([[0], np.cumsum(C)])
    total_chunks = int(off[-1])
    R = []
    cmaxm = cntm.max(axis=0)
    for m in range(NM):
        rt = []
        for c in range(C[m]):
            rows = int(min(max(cmaxm[m] - c * P, 0), P))
            rt.append(max(16, int(np.ceil(rows / 16) * 16)))
        R.append(rt)

    eidx = np.zeros((nco, P, total_chunks), np.int32)
    edst = np.full((nco, P, total_chunks), 3000.0, np.float32)
    order = np.lexsort((mslot, dmacro, dcore))
    s_sorted = gpad[src][order]
    k_sorted = dcore[order]
    m_sorted = dmacro[order]
    slot_sorted = mslot[order]
    grp = k_sorted * NM + m_sorted
    grp_change = np.concatenate([[True], grp[1:] != grp[:-1]])
    grp_first = np.where(grp_change)[0]
    grp_id = np.cumsum(grp_change) - 1
    pos = np.arange(len(order)) - grp_first[grp_id]
    col = off[m_sorted] + pos // P
    row = pos % P
    eidx[k_sorted, row, col] = s_sorted
    edst[k_sorted, row, col] = slot_sorted

    batchT = np.full((nco, P, NT), -1.0, np.float32)
    for k in range(nco):
        bl = batch[node_start[k]:node_start[k + 1]] - k * gpc
        blp = bl[invperm_by_core[k]]
        bt = np.full(PAD_N, -1.0, np.float32)
        bt[:counts[k]] = blp
        batchT[k] = bt.reshape(NT, P).T

    JB = int(np.ceil(gpc / P))
    TJ0 = np.full(JB, NT, np.int64)
    TJ1 = np.zeros(JB, np.int64)
    for k in range(nco):
        bl = batch[node_start[k]:node_start[k + 1]] - k * gpc
        for J in range(JB):
            lo = np.searchsorted(bl, J * P, side="left")
            hi = np.searchsorted(bl, min((J + 1) * P, gpc), side="left")
            if hi > lo:
                TJ0[J] = min(TJ0[J], lo // P)
                TJ1[J] = max(TJ1[J], (hi - 1) // P + 1)
    TJ0 = np.minimum(TJ0, TJ1)

    meta = dict(PAD_N=PAD_N, NT=NT, NM=NM, MW=MW, C=C.tolist(), off=off,
                total_chunks=total_chunks, R=R,
                JB=JB, TJ0=TJ0.tolist(), TJ1=TJ1.tolist(),
                node_start=node_start, counts=counts,
                invperm_by_core=invperm_by_core)
    percore = dict(eidx=eidx, edst=edst, batchT=batchT)
    return meta, percore


def fold_bn(w1b, gamma, beta, rmean, rvar):
    s = gamma / np.sqrt(rvar + BN_EPS)
    t = (w1b - rmean) * s + beta
    return s.astype(np.float32), t.astype(np.float32)


def build(cfg, meta):
    F, H = cfg.F, cfg.H
    NT, C, off = meta["NT"], meta["C"], meta["off"]
    R = meta["R"]
    NM, MW = meta["NM"], meta["MW"]
    W = MW * P
    PAD_N, TC = meta["PAD_N"], meta["total_chunks"]
    JB, TJ0, TJ1 = meta["JB"], meta["TJ0"], meta["TJ1"]
    nco = cfg.NCORES
    f32 = mybir.dt.float32
    bf = mybir.dt.float16

    nc = bacc.Bacc("TRN2", target_bir_lowering=False, debug=False, num_devices=nco,
                   enable_asserts=False)
    tc = tile.TileContext(nc, num_cores=nco)

    def dram_in(name, shape, dt=f32):
        return nc.dram_tensor(name, shape, dt, kind="ExternalInput").ap()

    xg = dram_in("xg", [nco * PAD_N, F], bf)
    x_local = dram_in("x_local", [PAD_N, F], bf)
    eidx = dram_in("eidx", [P, TC], mybir.dt.int32)
    edst = dram_in("edst", [P, TC])
    batchT = dram_in("batchT", [P, NT])
    w1_1 = dram_in("w1_1", [F, H], bf)
    w2 = {l: dram_in(f"w2_{l}", [H, H], bf) for l in (1, 2, 3)}
    w1n = {l: dram_in(f"w1n_{l}", [H, H], bf) for l in (2, 3)}
    bn_s = {l: dram_in(f"bn_s_{l}", [H, 1]) for l in (1, 2, 3)}
    bn_t = {l: dram_in(f"bn_t_{l}", [H, 1]) for l in (1, 2, 3)}
    b2 = {l: dram_in(f"b2_{l}", [H, 1]) for l in (1, 2)}
    b2row3 = dram_in("b2row3", [1, H], bf)
    wfc1 = dram_in("wfc1", [H, H // 2], bf)
    bfc1 = dram_in("bfc1", [H // 2, 1])
    wfc2 = dram_in("wfc2", [H // 2, 1], bf)
    bfc2 = dram_in("bfc2", [1, 1])

    out = nc.dram_tensor("out", [1, JB * P], f32, kind="ExternalOutput").ap()

    RELU = mybir.ActivationFunctionType.Relu
    IDENT = mybir.ActivationFunctionType.Identity

    with tc:
        with (
            tc.tile_pool(name="const", bufs=1) as cpool,
            tc.tile_pool(name="gat", bufs=24) as gpool,
            tc.tile_pool(name="smat", bufs=6) as spool,
            tc.tile_pool(name="work", bufs=3) as wpool,
            tc.tile_pool(name="yout", bufs=4) as ypool,
            tc.tile_pool(name="psum", bufs=2, space="PSUM") as pspool,
            tc.tile_pool(name="psy", bufs=3, space="PSUM") as psy,
            tc.tile_pool(name="pool_ps", bufs=3, space="PSUM") as ppool,
            tc.tile_pool(name="dram", bufs=1, space="DRAM") as dpool,
        ):
            # ---- constants ----
            iota_i = cpool.tile([P, W], mybir.dt.int32)
            nc.gpsimd.iota(iota_i[:], pattern=[[1, W]], base=0, channel_multiplier=0)
            iota_f = cpool.tile([P, W], f32)
            nc.vector.tensor_copy(iota_f[:], iota_i[:])
            ident = cpool.tile([P, P], bf)
            make_identity(nc, ident[:])
            ones_row = cpool.tile([1, P], bf)
            nc.vector.memset(ones_row[:], 1.0)

            eidx_sb = cpool.tile([P, TC], mybir.dt.int32)
            nc.sync.dma_start(eidx_sb[:], eidx[:, :])
            edst_sb = cpool.tile([P, TC], f32)
            nc.sync.dma_start(edst_sb[:], edst[:, :])
            batch_sb = cpool.tile([P, NT], f32)
            nc.sync.dma_start(batch_sb[:], batchT[:, :])

            w1_1_sb = cpool.tile([F, H], bf)
            nc.sync.dma_start(w1_1_sb[:], w1_1[:, :])
            w2_sb, w1n_sb, bns_sb, bnt_sb, b2_sb = {}, {}, {}, {}, {}
            for l in (1, 2, 3):
                w2_sb[l] = cpool.tile([H, H], bf, tag=f"w2_{l}", name=f"w2sb_{l}")
                nc.sync.dma_start(w2_sb[l][:], w2[l][:, :])
                bns_sb[l] = cpool.tile([H, 1], f32, tag=f"bns_{l}", name=f"bnssb_{l}")
                nc.sync.dma_start(bns_sb[l][:], bn_s[l][:, :])
                bnt_sb[l] = cpool.tile([H, 1], f32, tag=f"bnt_{l}", name=f"bntsb_{l}")
                nc.sync.dma_start(bnt_sb[l][:], bn_t[l][:, :])
            for l in (2, 3):
                w1n_sb[l] = cpool.tile([H, H], bf, tag=f"w1n_{l}", name=f"w1nsb_{l}")
                nc.sync.dma_start(w1n_sb[l][:], w1n[l][:, :])
            for l in (1, 2):
                b2_sb[l] = cpool.tile([H, 1], f32, tag=f"b2_{l}", name=f"b2sb_{l}")
                nc.sync.dma_start(b2_sb[l][:], b2[l][:, :])
            b2row3_sb = cpool.tile([1, H], bf)
            nc.sync.dma_start(b2row3_sb[:], b2row3[:, :])
            wfc1_sb = cpool.tile([H, H // 2], bf)
            nc.sync.dma_start(wfc1_sb[:], wfc1[:, :])
            bfc1_sb = cpool.tile([H // 2, 1], f32)
            nc.sync.dma_start(bfc1_sb[:], bfc1[:, :])
            wfc2_sb = cpool.tile([H // 2, 1], bf)
            nc.sync.dma_start(wfc2_sb[:], wfc2[:, :])
            bfc2_sb = cpool.tile([1, 1], f32)
            nc.sync.dma_start(bfc2_sb[:], bfc2[:, :])

            y_in = {l: dpool.tile([PAD_N, H], bf, tag=f"y_in_{l}", name=f"y_in_{l}")
                    for l in (2, 3)}
            y_g = {l: dpool.tile([nco * PAD_N, H], bf, tag=f"y_g_{l}", name=f"y_g_{l}",
                                 addr_space="Shared") for l in (2, 3)}

            pool_tiles = {}

            def agg_psum(m, gather_dram, self_dram, zdim):
                """Aggregate one 512-slot macro window into PSUM [zdim, W]."""
                zp = pspool.tile([zdim, W], f32, tag="zps", name=f"zp{m}")
                nchunk = C[m]
                subs = [t for t in range(m * MW, min((m + 1) * MW, NT))]
                for c in range(nchunk):
                    cc = off[m] + c
                    r = R[m][c]
                    s = spool.tile([P, W], bf, tag="s", name=f"s{m}_{c}")
                    nc.vector.tensor_tensor(
                        out=s[:], in0=edst_sb[:, cc:cc + 1].to_broadcast([P, W]),
                        in1=iota_f[:], op=mybir.AluOpType.is_equal)
                    g = gpool.tile([P, zdim], bf, tag="g", name=f"g{m}_{c}")
                    nc.gpsimd.indirect_dma_start(
                        out=g[:r, :], out_offset=None, in_=gather_dram[:],
                        in_offset=bass.IndirectOffsetOnAxis(
                            ap=eidx_sb[:r, cc:cc + 1], axis=0),
                    )
                    nc.tensor.matmul(out=zp[:], lhsT=g[:r, :], rhs=s[:r, :],
                                     start=(c == 0), stop=False)
                for si, t in enumerate(subs):
                    sf = gpool.tile([P, zdim], bf, tag="sf", name=f"sf{m}_{si}")
                    nc.sync.dma_start(sf[:], self_dram[t * P:(t + 1) * P, :])
                    s4 = t - m * MW
                    nc.tensor.matmul(out=zp[:, s4 * P:(s4 + 1) * P], lhsT=sf[:],
                                     rhs=ident[:], start=False,
                                     stop=(si == len(subs) - 1))
                return zp

            def store_y(l, t, ynp):
                ysb = ypool.tile([P, H], bf, tag="ysb", name=f"ysb{l}_{t}")
                nc.vector.tensor_copy(ysb[:], ynp[:])
                nc.sync.dma_start(y_in[l][t * P:(t + 1) * P, :], ysb[:])

            # ================= layer 1 =================
            for m in range(NM):
                subs = [t for t in range(m * MW, min((m + 1) * MW, NT))]
                zp = agg_psum(m, xg, x_local, F)
                zx = wpool.tile([F, W], bf, tag="zx", name=f"zx{m}")
                nc.vector.tensor_copy(zx[:], zp[:])
                hp = psy.tile([H, W], f32, tag="mm", name=f"hp{m}")
                nc.tensor.matmul(out=hp[:], lhsT=w1_1_sb[:], rhs=zx[:],
                                 start=True, stop=True)
                h1 = wpool.tile([H, W], bf, tag="h1", name=f"h1a{m}")
                nc.scalar.activation(out=h1[:], in_=hp[:], func=RELU,
                                     bias=bnt_sb[1][:, :1], scale=bns_sb[1][:, :1])
                h2p = psy.tile([H, W], f32, tag="mm", name=f"h2p{m}")
                nc.tensor.matmul(out=h2p[:], lhsT=w2_sb[1][:], rhs=h1[:],
                                 start=True, stop=True)
                h1f = wpool.tile([H, W], bf, tag="h1f", name=f"h1fa{m}")
                nc.scalar.activation(out=h1f[:], in_=h2p[:], func=RELU,
                                     bias=b2_sb[1][:, :1], scale=1.0)
                for t in subs:
                    s4 = t - m * MW
                    ynp = psy.tile([P, H], f32, tag="mm", name=f"ynp{t}")
                    nc.tensor.matmul(out=ynp[:], lhsT=h1f[:, s4 * P:(s4 + 1) * P],
                                     rhs=w1n_sb[2][:], start=True, stop=True)
                    store_y(2, t, ynp)

            nc.gpsimd.collective_compute(
                "AllGather", mybir.AluOpType.bypass,
                replica_groups=[list(range(nco))],
                ins=[y_in[2][:].opt()], outs=[y_g[2][:].opt()])

            # ================= layer 2 =================
            for m in range(NM):
                subs = [t for t in range(m * MW, min((m + 1) * MW, NT))]
                zp = agg_psum(m, y_g[2], y_in[2], H)
                h1 = wpool.tile([H, W], bf, tag="h1", name=f"h1b{m}")
                nc.scalar.activation(out=h1[:], in_=zp[:], func=RELU,
                                     bias=bnt_sb[2][:, :1], scale=bns_sb[2][:, :1])
                h2p = psy.tile([H, W], f32, tag="mm", name=f"h2pb{m}")
                nc.tensor.matmul(out=h2p[:], lhsT=w2_sb[2][:], rhs=h1[:],
                                 start=True, stop=True)
                h1f = wpool.tile([H, W], bf, tag="h1f", name=f"h1fb{m}")
                nc.scalar.activation(out=h1f[:], in_=h2p[:], func=RELU,
                                     bias=b2_sb[2][:, :1], scale=1.0)
                for t in subs:
                    s4 = t - m * MW
                    ynp = psy.tile([P, H], f32, tag="mm", name=f"ynpb{t}")
                    nc.tensor.matmul(out=ynp[:], lhsT=h1f[:, s4 * P:(s4 + 1) * P],
                                     rhs=w1n_sb[3][:], start=True, stop=True)
                    store_y(3, t, ynp)

            nc.gpsimd.collective_compute(
                "AllGather", mybir.AluOpType.bypass,
                replica_groups=[list(range(nco))],
                ins=[y_in[3][:].opt()], outs=[y_g[3][:].opt()])

            # ================= layer 3 + pooling + head =================
            for m in range(NM):
                subs = [t for t in range(m * MW, min((m + 1) * MW, NT))]
                zp = agg_psum(m, y_g[3], y_in[3], H)
                h1w = wpool.tile([H, W], bf, tag="h1", name=f"h1c{m}")
                nc.scalar.activation(out=h1w[:], in_=zp[:], func=RELU,
                                     bias=bnt_sb[3][:, :1], scale=bns_sb[3][:, :1])
              # per-subtile tail
                for t in subs:
                    s4 = t - m * MW
                    h3p = psy.tile([P, H], f32, tag="mm", name=f"h3p{t}")
                    nc.tensor.matmul(out=h3p[:], lhsT=h1w[:, s4 * P:(s4 + 1) * P],
                                     rhs=w2_sb[3][:], start=True, stop=False)
                    nc.tensor.matmul(out=h3p[:], lhsT=ones_row[:], rhs=b2row3_sb[:],
                                     start=False, stop=True)
                    h3t = wpool.tile([P, H], bf, tag="h3t", name=f"h3t{t}")
                    nc.scalar.activation(out=h3t[:], in_=h3p[:], func=RELU)
                    for J in range(JB):
                        if not (TJ0[J] <= t < TJ1[J]):
                            continue
                        sg = spool.tile([P, P], bf, tag="sg", name=f"sg{t}_{J}")
                        nc.vector.scalar_tensor_tensor(
                            out=sg[:], in0=batch_sb[:, t:t + 1].to_broadcast([P, P]),
                            scalar=float(J * P), op0=mybir.AluOpType.subtract,
                            in1=iota_f[:, :P], op1=mybir.AluOpType.is_equal)
                        if J not in pool_tiles:
                            pool_tiles[J] = ppool.tile([H, P], f32, tag="plp",
                                                       name=f"plt{J}")
                        nc.tensor.matmul(
                            out=pool_tiles[J][:], lhsT=h3t[:], rhs=sg[:],
                            start=(t == TJ0[J]), stop=(t == TJ1[J] - 1))
                        if t == TJ1[J] - 1:
                            pool_sb = wpool.tile([H, P], bf, tag="pool_sb",
                                                 name=f"pool_sb{J}")
                            nc.vector.tensor_copy(pool_sb[:], pool_tiles[J][:])
                            f1p = psy.tile([H // 2, P], f32, tag="mm", name=f"f1p{J}")
                            nc.tensor.matmul(out=f1p[:], lhsT=wfc1_sb[:], rhs=pool_sb[:],
                                             start=True, stop=True)
                            f1 = wpool.tile([H // 2, P], bf, tag="f1", name=f"f1{J}")
                            nc.scalar.activation(out=f1[:], in_=f1p[:], func=RELU,
                                                 bias=bfc1_sb[:, :1], scale=1.0)
                            f2p = psy.tile([1, P], f32, tag="mm", name=f"f2p{J}")
                            nc.tensor.matmul(out=f2p[:], lhsT=wfc2_sb[:], rhs=f1[:],
                                             start=True, stop=True)
                            ojt = ypool.tile([1, P], f32, tag="ojt", name=f"ojt{J}")
                            nc.scalar.activation(out=ojt[:], in_=f2p[:], func=IDENT,
                                                 bias=bfc2_sb[:, :1], scale=1.0)
                            nc.sync.dma_start(out[:1, J * P:(J + 1) * P], ojt[:])

    nc.finalize()
    return nc


def make_in_maps(cfg, meta, percore, x, weights):
    nco = cfg.NCORES
    PAD_N = meta["PAD_N"]
    node_start, counts = meta["node_start"], meta["counts"]
    invperm = meta["invperm_by_core"]
    F, H = cfg.F, cfg.H

    xg = np.zeros((nco * PAD_N, F), BF16)
    xls = []
    for k in range(nco):
        xs = x[node_start[k]:node_start[k + 1]][invperm[k]]
        xg[k * PAD_N:k * PAD_N + counts[k]] = xs.astype(BF16)
        xl = np.zeros((PAD_N, F), BF16)
        xl[:counts[k]] = xs.astype(BF16)
        xls.append(xl)

    w = {k: np.asarray(v, np.float32) for k, v in weights.items()}
    folded = {}
    for l in (1, 2, 3):
        s, t = fold_bn(w[f"b1_{l}"], w[f"gamma_{l}"], w[f"beta_{l}"],
                       w[f"rmean_{l}"], w[f"rvar_{l}"])
        folded[f"bn_s_{l}"] = s.reshape(H, 1)
        folded[f"bn_t_{l}"] = t.reshape(H, 1)

    common = dict(
        xg=xg,
        w1_1=w["w1_1"].astype(BF16),
        w2_1=w["w2_1"].astype(BF16), w2_2=w["w2_2"].astype(BF16),
        w2_3=w["w2_3"].astype(BF16),
        w1n_2=w["w1_2"].astype(BF16), w1n_3=w["w1_3"].astype(BF16),
        b2_1=w["b2_1"].reshape(H, 1), b2_2=w["b2_2"].reshape(H, 1),
        b2row3=w["b2_3"].reshape(1, H).astype(BF16),
        wfc1=w["w_fc1"].astype(BF16), bfc1=w["b_fc1"].reshape(H // 2, 1),
        wfc2=w["w_fc2"].astype(BF16), bfc2=w["b_fc2"].reshape(1, 1),
        **folded,
    )
    in_maps = []
    for k in range(nco):
        in_maps.append(dict(
            common,
            x_local=xls[k],
            eidx=percore["eidx"][k],
            edst=percore["edst"][k],
            batchT=percore["batchT"][k],
        ))
    return in_maps


def assemble_output(cfg, results):
    outs = []
    for k in range(cfg.NCORES):
        outs.append(results[k]["out"][0, :cfg.G_PER_CORE])
    return np.concatenate(outs).reshape(cfg.G, 1).astype(np.float32)


# ============================================================================
# Self-contained kernel entry point
# ============================================================================

N_NODES = 200000
N_EDGES = 600000
N_GRAPHS = 10000
F_NODE = 32
H_DIM = 128
N_CORES = 8

_CACHE = {}

_WEIGHT_KEYS = tuple(
    f"{p}_{l}" for l in (1, 2, 3)
    for p in ("w1", "b1", "gamma", "beta", "rmean", "rvar", "w2", "b2")
) + ("w_fc1", "b_fc1", "w_fc2", "b_fc2")


def kernel(**inputs):
    """Full-input GIN GNN forward on 8 TRN2 NeuronCores.

    Takes the unsharded inputs of reference.setup_inputs(), distributes the
    graph across 8 cores internally, and returns the [N_GRAPHS, 1] float32
    output. edge_attr only feeds a dead branch of the reference and is unused.
    """
    x = np.asarray(inputs["x"], np.float32)
    edge_index = np.asarray(inputs["edge_index"])
    batch = np.asarray(inputs["batch"])
    weights = {k: np.asarray(inputs[k], np.float32) for k in _WEIGHT_KEYS}

    cfg = Cfg(N_NODES, N_EDGES, N_GRAPHS, F_NODE, H_DIM, N_CORES)
    key = (edge_index.tobytes(), batch.tobytes())
    ck = _CACHE.get("graph_key")
    if ck != key:
        meta, percore = preprocess_graph(cfg, edge_index, batch)
        nc = build(cfg, meta)
        _CACHE.update(graph_key=key, meta=meta, percore=percore, nc=nc)
    meta, percore, nc = _CACHE["meta"], _CACHE["percore"], _CACHE["nc"]

    in_maps = make_in_maps(cfg, meta, percore, x, weights)
    res = bass_utils.run_bass_kernel_spmd(nc, in_maps, core_ids=list(range(N_CORES)))
    return assemble_output(cfg, res.results)


def run_traced(**inputs):
    """Like kernel() but with NTFF tracing; returns (output, exec_time_ns)."""
    import types as _types

    def _install_hook_shim():
        import antenv
        if "antenv.axon_hooks" in sys.modules:
            return
        try:
            from trn_agent_boot.trn_boot import _ntff_profile_via_ctypes
            hook = _ntff_profile_via_ctypes("/opt/axon/libaxon_pjrt.so")
        except Exception:
            hook = None
        mod = _types.ModuleType("antenv.axon_hooks")
        mod.get_axon_ntff_profile_hook = lambda: hook
        mod.set_axon_ntff_profile_hook = lambda h: None
        sys.modules["antenv.axon_hooks"] = mod
        antenv.axon_hooks = mod

    _install_hook_shim()
    import tempfile
    x = np.asarray(inputs["x"], np.float32)
    edge_index = np.asarray(inputs["edge_index"])
    batch = np.asarray(inputs["batch"])
    weights = {k: np.asarray(inputs[k], np.float32) for k in _WEIGHT_KEYS}
    cfg = Cfg(N_NODES, N_EDGES, N_GRAPHS, F_NODE, H_DIM, N_CORES)
    meta, percore = preprocess_graph(cfg, edge_index, batch)
    nc = build(cfg, meta)
    in_maps = make_in_maps(cfg, meta, percore, x, weights)
    tmpdir = tempfile.mkdtemp(prefix="gnn_ntff_")
    res = bass_utils.run_bass_kernel_spmd(nc, in_maps, core_ids=list(range(N_CORES)),
                                          trace=True, tmpdir=tmpdir)
    return assemble_output(cfg, res.results), res.exec_time_ns

